# revision 56
# baseline (speedup 1.0000x reference)
"""Trainium2 Bass kernel for GroupNorm + spatial self-attention + residual.

Reference computation (B=1, C=512, H=W=64, 8 heads x 64 dim, GN groups=32):
    x = GroupNorm(hidden_states) -> tokens [N=4096, C]
    q,k,v = x @ {wq,wk,wv}.T  (per-head slices of inner=512)
    out = softmax(q k^T / 8) v   per head
    y = concat_heads(out) @ wo.T + bo + hidden_states

Distribution over 8 NeuronCores: head-parallel attention (core h owns head h;
every core reads the full input), then an AllToAll that token-shards the
attention output so core j computes the output projection + bias + residual
for tokens [512j, 512j+512) only.

Per-core device graph (SPMD, same graph on all 8 cores, per-core data differs):
  1. DMA x [512, 4096] f32 into SBUF; xb = raw bf16 cast on ScalarE (no
     affine) overlapped with per-channel stats on VectorE; per-tile group
     stats via block-ones matmuls; rstd = exp(-0.5 ln(var+eps)).
  2. The GroupNorm affine is FOLDED INTO THE PROJECTION WEIGHTS:
     w' = w * s per channel (cast bf16), plus rank-1 bias terms
     qb = wq^T b (applied in the psum->sbuf copy via Identity+bias) and
     vb = b^T wv (applied as an extra K=1 matmul in the v accumulation).
     The K-side bias is DROPPED: its score term q.kb is constant along the
     softmax (key) axis, so softmax is unchanged.
  3. Row-packed layouts for tile_position score matmuls: qTd [128, 4096]
     holds q^T duplicated in both partition halves; kTd [128, 16, 128]
     holds key chunks 0-15 (tokens 0-2047) in partitions 0-63 and chunks
     16-31 in partitions 64-127 (pair j = chunks (j, j+16); softmax is
     permutation-invariant over keys, and this HALF-PAIRING keeps every
     psum->SBUF copy contiguous). v in fp8e4 [128, parity, pair, 80]:
     cols 0-63 = v (GN bias included), col 64 = ones -- emitted by the
     host-zero-padded wv plus vb[64]=1 through the rank-1 bias matmul.
  4. Flash-style attention in transposed layout: the two 128-key chunks of
     a pair run CONCURRENTLY as 64-row tile_position row-tiles (K=64 only
     fills half the PE array; packing doubles score throughput) into the
     two banks of a pair-sized psum tile; ONE exp instruction per 256-key
     pair (alternating ScalarE Exp / VectorE EXP16, fp8 out); PV runs fp8
     DoubleRow with the 80-col padded stationary so rows 0-63 accumulate
     PV and row 64 accumulates the softmax denominator IN THE SAME MATMUL
     (the 80-col pad satisfies the dual-fp8 ldweights 16B step rule).
     No max-subtraction (scores are O(1) by construction).
  5. Unnormalized out [65, 512] per query block (row 64 = denominators),
     one ScalarE copy, bf16 -> DRAM -> AllToAll.
  6. Each core normalizes its received token chunk (ln/exp reciprocal,
     batched gather + broadcast DMAs), output projection (i-outer matmuls),
     +bo, +residual, writes its [512, 512] column chunk of the output.
"""

import sys

sys.path.insert(0, "/opt/trn_rl_repo")

import numpy as np

import concourse.bacc as bacc
import concourse.tile as tile
from concourse import mybir
from concourse.bass_utils import run_bass_kernel_spmd

C = 512
N = 4096
HEADS = 8
D = 64
GROUPS = 32
CPG = C // GROUPS  # 16 channels per group
EPS = 1e-5
SCALE = D ** -0.5
NCORE = 8
NT = N // NCORE  # 512 tokens per core for the output projection
TQ = 512  # query-chunk (free dim of transposed scores); one psum bank
NTQ = N // TQ  # 8
TKC = 128  # key-chunk (partition dim of transposed scores)
NTK = N // TKC  # 32
CT = C // 128  # 4 channel tiles
DP = 80  # fp8 V stationary padded to 80 cols: col 64 = ones (denominator),
#          cols 65-79 = zero pad so the DoubleRow weights step is 16B-aligned

f32 = mybir.dt.float32
bf16 = mybir.dt.bfloat16
f8 = mybir.dt.float8e4
AF = mybir.ActivationFunctionType
ALU = mybir.AluOpType

_nc_cache = {}

# exp(SCALE*x) ~= ((x*EC0 + EC1)^2 + 0.5)^16  -- a (1 + y/16 + y^2/512)^16
# approximation computed in one fused VectorE pass (8 ALU stages), used to
# split softmax exp work between ScalarE and VectorE. Max rel err 2.9e-3 at
# |y|=1.6 (scores here stay well inside that), 3.5e-4 for |y|<0.8.
EC0 = SCALE / float(np.sqrt(512.0))
EC1 = float(np.sqrt(0.5))
SKEW_PAIRS = 3  # PV consumes pair j while scores/exp work on pair j+SKEW

# ACT/DVE exp split, one entry per 256-key PAIR: measured HW per-pair costs
# are near-equal (ScalarE ~1.68us vs VectorE-EXP16 ~1.72us incl. overhead),
# so strict alternation balances load and keeps consecutive pairs on
# different engines for maximum overlap.
EXP_DVE_PAT = tuple(j % 2 == 1 for j in range(NTK))


def _register_exp16():
    from concourse import dve_ops as dops
    from concourse.dve_spec import Spec, Src0, C0, C1, sq

    for op in dops.OPS:
        if op.name == "EXP16_ANT":
            return op
    t = sq(Src0 * C0 + C1) + C2_LEAF
    body = sq(sq(sq(sq(t))))
    spec = Spec(
        body=body,
        reference=lambda in0, in1, s0, s1, imm2: ((in0 * s0 + s1) ** 2 + imm2)
        ** 16,
    )
    op = dops.DveOp("EXP16_ANT", spec, subdim=False, uops_sha={})
    dops.OPS.append(op)
    dops.CUSTOM_DVE_SPECS[op.name] = op.spec
    dops._SUB_OPCODE_FOR_NAME[op.name] = dops._CUSTOM_DVE_ROW_BASE + len(dops.OPS) - 1
    from concourse.dve_uop import DveOpSpec
    from concourse.dve_spec import lower as dve_lower

    for ver in ("v3", "v4"):
        try:
            uops = dve_lower(spec, ver=ver)
            sha = DveOpSpec(
                name=op.name,
                opcode=dops.get_dve_sub_opcode(op.name),
                uops=uops,
                rd1_en=False,
            ).sha(ver)
            op.uops_sha[ver] = sha
        except Exception:
            pass
    return op


from concourse.dve_spec import C2 as C2_LEAF  # noqa: E402

EXP16 = _register_exp16()


def _attention_stage(nc, tc, ps_s, ps_o, pp, kTd, qTd, vpair_all, a2a_in):
    NPAIR = NTK // 2  # 16 pairs of 128-key chunks; PV runs fp8 DoubleRow
    PM = mybir.MatmulPerfMode.DoubleRow
    # exp tiles cover ETC=3 key chunks each (11 exp instructions per query
    # block instead of 16 -- the ~0.5us fixed cost per exp instruction
    # dominates). The exp'd probabilities land in ONE contiguous fp8 buffer
    # per query block so the PV DoubleRow pair APs are unaffected; Tile's
    # subtile dependency tracking orders PV(j) after every exp tile that
    # covers chunks 2j / 2j+1.
    ETC = 3
    tiles = []  # (chunk_lo, chunk_hi) per exp tile
    c = 0
    while c < NTK:
        tiles.append((c, min(c + ETC, NTK)))
        c += ETC
    NTILE = len(tiles)
    SKEW_T = 2  # PV trails the exp tiles by this many tiles

    for jq in range(NTQ):
        # merged PV+den DoubleRow output: rows 0-63 = PV, row 64 = the ones
        # column denominator, rows 65-79 = padding (the fp8 weights are
        # padded to 80 cols so the dual-fp8 ldweights step%16==0 rule holds).
        ops = ps_o.tile([DP, TQ], f32, name="ops", tag="ops")
        pbig = pp.tile([128, NTK, TQ], f8, name="pbig", tag="pbig", bufs=1)
        pv_done = [0]  # next PV pair index to issue

        def do_tile(t):
            # pbig position p holds chunk (p//2) + 16*(p%2): pair j is the
            # contiguous positions (2j, 2j+1), and adjacent positions sit in
            # DIFFERENT kTd partition halves, so consecutive score matmuls
            # run concurrently in the two tile_position row groups (K=64
            # only fills half the PE rows; packing doubles throughput).
            lo, hi = tiles[t]
            sps = ps_s.tile(
                [128, hi - lo, TQ], f32, name="sps", tag="sps", bufs=2
            )
            for pos in range(lo, hi):
                half = pos % 2
                slot = pos // 2
                base = 64 * half
                nc.tensor.matmul(
                    sps[:, pos - lo, :],
                    kTd[base : base + 64, slot, :],
                    qTd[base : base + 64, jq * TQ : (jq + 1) * TQ],
                    start=True,
                    stop=True,
                )
            if EXP_DVE_PAT[t % len(EXP_DVE_PAT)]:
                nc.vector._custom_dve(
                    EXP16, out=pbig[:, lo:hi, :], in0=sps[:, :, :],
                    s0=EC0, s1=EC1, imm2=0.5,
                )
            else:
                nc.scalar.activation(
                    pbig[:, lo:hi, :], sps[:, :, :], AF.Exp, scale=SCALE
                )

        def pv_upto(pos_hi):
            # pair j is ready once positions 2j and 2j+1 are both exp'd
            while pv_done[0] < NPAIR and 2 * pv_done[0] + 1 < pos_hi:
                j = pv_done[0]
                pv_done[0] += 1
                nc.tensor.matmul(
                    ops[:, :],
                    vpair_all[:, :, j, :],
                    pbig[:, 2 * j : 2 * j + 2, :],
                    start=(j == 0),
                    stop=(j == NPAIR - 1),
                    perf_mode=PM,
                )

        for t in range(NTILE):
            do_tile(t)
            if t >= SKEW_T:
                pv_upto(tiles[t - SKEW_T][1])
        pv_upto(NTK)

        o_sb = pp.tile([D + 1, TQ], bf16, name="o_sb", tag="o_sb", bufs=2)
        if jq % 2 == 0:
            nc.scalar.activation(o_sb[:, :], ops[0 : D + 1, :], AF.Copy)
        else:
            nc.vector.tensor_copy(o_sb[:, :], ops[0 : D + 1, :])
        nc.sync.dma_start(a2a_in[jq, :, :], o_sb[:, :])


def _build(attn_loop_k=None, pre_loop_k=None, tail_loop_k=None, pre_scope="all"):
    import contextlib
    nc = bacc.Bacc("TRN2", target_bir_lowering=False, debug=False, num_devices=NCORE)

    def scope_cm(tc, name):
        # For_i around just one pre sub-stage when pre_scope selects it
        if pre_loop_k and pre_scope == name:
            return tc.For_i(0, pre_loop_k, 1)
        return contextlib.nullcontext()

    x_d = nc.dram_tensor("x", [C, N], f32, kind="ExternalInput")
    gamma_d = nc.dram_tensor("gamma", [C, 1], f32, kind="ExternalInput")
    beta_d = nc.dram_tensor("beta", [C, 1], f32, kind="ExternalInput")
    wqT_d = nc.dram_tensor("wqT", [C, D], f32, kind="ExternalInput")
    wkT_d = nc.dram_tensor("wkT", [C, D], f32, kind="ExternalInput")
    # wv arrives zero-padded to [C, DP]: the v matmuls then emit the padded
    # 80-col layout directly and the ones (denominator) column comes from
    # the rank-1 bias matmul with vb[64] overwritten to 1.0.
    wvT_d = nc.dram_tensor("wvT", [C, DP], f32, kind="ExternalInput")
    woT_d = nc.dram_tensor("woT", [C, C], f32, kind="ExternalInput")
    bo_d = nc.dram_tensor("bo", [C, 1], f32, kind="ExternalInput")
    resid_d = nc.dram_tensor("resid", [C, NT], f32, kind="ExternalInput")
    bones_d = nc.dram_tensor("bones", [128, 8], f32, kind="ExternalInput")
    bonesT_d = nc.dram_tensor("bonesT", [8, 128], f32, kind="ExternalInput")
    out_d = nc.dram_tensor("out", [C, NT], f32, kind="ExternalOutput")

    with tile.TileContext(nc) as tc:
        with (
            tc.tile_pool(name="xc", bufs=1) as pxc,
            tc.tile_pool(name="xb", bufs=1) as pxb,
            tc.tile_pool(name="qk", bufs=1) as pqk,
            tc.tile_pool(name="vaug", bufs=1) as pva,
            tc.tile_pool(name="w", bufs=1) as pw,
            tc.tile_pool(name="small", bufs=1) as psm,
            tc.tile_pool(name="p", bufs=3) as pp,
            tc.tile_pool(name="post", bufs=1) as ppost,
            tc.tile_pool(name="dram", bufs=1, space="DRAM") as pdram,
        ):
            # ---------------- stage 1: load x + GroupNorm statistics ----------
            pre_cm = (
                tc.For_i(0, pre_loop_k, 1)
                if pre_loop_k and pre_scope == "all"
                else contextlib.nullcontext()
            )
            pre_cm.__enter__()
            s1_cm = scope_cm(tc, "s1")
            s1_cm.__enter__()
            # ---- x DMAs first (the long pole), params after ----------------
            xc = [pxc.tile([128, N], f32, name=f"xc{i}") for i in range(CT)]
            for i in range(CT):
                nc.sync.dma_start(xc[i][:, :], x_d[i * 128 : (i + 1) * 128, :])
            bones = psm.tile([128, 8], f32, name="bones")
            bonesT = psm.tile([8, 128], f32, name="bonesT")
            gamma_sb = psm.tile([128, CT], f32, name="gamma_sb")
            beta_sb = psm.tile([128, CT], f32, name="beta_sb")
            nc.sync.dma_start(bones[:, :], bones_d[:, :])
            nc.sync.dma_start(bonesT[:, :], bonesT_d[:, :])
            nc.sync.dma_start(
                gamma_sb[:, :], gamma_d.rearrange("(t p) o -> p (t o)", p=128)
            )
            nc.sync.dma_start(
                beta_sb[:, :], beta_d.rearrange("(t p) o -> p (t o)", p=128)
            )
            wq_sb = [pw.tile([128, D], f32, name=f"wq{i}") for i in range(CT)]
            wk_sb = [pw.tile([128, D], f32, name=f"wk{i}") for i in range(CT)]
            wv_sb = [pw.tile([128, DP], f32, name=f"wv{i}") for i in range(CT)]
            for i in range(CT):
                nc.sync.dma_start(wq_sb[i][:, :], wqT_d[i * 128 : (i + 1) * 128, :])
                nc.sync.dma_start(wk_sb[i][:, :], wkT_d[i * 128 : (i + 1) * 128, :])
                nc.sync.dma_start(wv_sb[i][:, :], wvT_d[i * 128 : (i + 1) * 128, :])

            # ---- per tile: raw bf16 cast (ACT) + channel stats (DVE) +
            # ---- per-tile group sums (PE): GN affine is folded into the
            # ---- projection weights later, so the cast needs no stats.
            xb = [pxb.tile([128, N], bf16, name=f"xb{i}") for i in range(CT)]
            cstat = [psm.tile([128, 2], f32, name=f"cs{i}") for i in range(CT)]
            gstat = psm.tile([8, 2, CT], f32, name="gstat")
            s_c = [psm.tile([128, 1], f32, name=f"s_c{i}") for i in range(CT)]
            b_c = [psm.tile([128, 1], f32, name=f"b_c{i}") for i in range(CT)]
            wqb = [pw.tile([128, D], bf16, name=f"wqb{i}") for i in range(CT)]
            wkb = [pw.tile([128, D], bf16, name=f"wkb{i}") for i in range(CT)]
            wvb = [pw.tile([128, DP], bf16, name=f"wvb{i}") for i in range(CT)]
            qb_sb = psm.tile([D, 1], f32, name="qb_sb")
            vb_sb = psm.tile([1, DP], bf16, name="vb_sb")
            ones_tok = psm.tile([1, 128], bf16, name="ones_tok")
            nc.vector.memset(ones_tok[:, :], 1.0)

            with tc.tile_pool(name="ps_pre", bufs=1, space="PSUM") as ps_pre:
                for i in range(CT):
                    # channel sums ride the bf16 cast for free (ACT accum);
                    # channel sum-of-squares is ONE big DVE pass per tile
                    # (bn_stats would need 8 instructions at the 512-free cap,
                    # and per-instruction overhead dominates on HW).
                    sx = psm.tile([128, 1], f32, name="sx", tag="sx", bufs=2)
                    nc.scalar.activation(
                        xb[i][:, :], xc[i][:, :], AF.Copy, accum_out=sx[:, :]
                    )
                    sq_scr = psm.tile([128, N], bf16, name="sq_scr", tag="sq", bufs=1)
                    sxx = psm.tile([128, 1], f32, name="sxx", tag="sxx", bufs=2)
                    nc.vector.scalar_tensor_tensor(
                        out=sq_scr[:, :], in0=xc[i][:, :], scalar=1.0,
                        in1=xc[i][:, :], op0=ALU.mult, op1=ALU.mult,
                        accum_out=sxx[:, :],
                    )
                    # cstat = (mean, E[x^2]) per channel
                    nc.vector.tensor_scalar_mul(cstat[i][:, 0:1], sx[:, :], 1.0 / N)
                    nc.vector.tensor_scalar_mul(cstat[i][:, 1:2], sxx[:, :], 1.0 / N)
                    gps = ps_pre.tile([8, 2], f32, name="gps", tag="gps", bufs=2)
                    nc.tensor.matmul(gps[:, :], bones[:, :], cstat[i][:, :])
                    nc.vector.tensor_copy(gstat[:, :, i], gps[:, :])

                # group mean/ex2 -> mean, rstd (layout [group8, stat2, tile4])
                gm = psm.tile([8, 2, CT], f32, name="gm")
                nc.vector.tensor_scalar_mul(gm[:, :, :], gstat[:, :, :], 1.0 / CPG)
                vtmp = psm.tile([8, 1, CT], f32, name="vtmp")
                nc.vector.tensor_mul(vtmp[:, 0, :], gm[:, 0, :], gm[:, 0, :])
                varg = psm.tile([8, 1, CT], f32, name="varg")
                nc.vector.tensor_sub(varg[:, 0, :], gm[:, 1, :], vtmp[:, 0, :])
                eps_sb = psm.tile([8, 1], f32, name="eps_sb")
                nc.vector.memset(eps_sb[:, :], EPS)
                lng = psm.tile([8, 1, CT], f32, name="lng")
                nc.scalar.activation(lng[:, 0, :], varg[:, 0, :], AF.Ln, bias=eps_sb[:, :])
                rstd = psm.tile([8, 1, CT], f32, name="rstd")
                nc.scalar.activation(rstd[:, 0, :], lng[:, 0, :], AF.Exp, scale=-0.5)
                gs2 = psm.tile([8, 2, CT], f32, name="gs2")
                nc.vector.tensor_copy(gs2[:, 0, :], gm[:, 0, :])
                nc.vector.tensor_copy(gs2[:, 1, :], rstd[:, 0, :])

                # expand group (mean, rstd) -> per-channel via bonesT matmul;
                # fold the GN affine into the projection weights
                for i in range(CT):
                    cbp = ps_pre.tile([128, 2], f32, name="cbp", tag="cbp", bufs=2)
                    nc.tensor.matmul(cbp[:, :], bonesT[:, :], gs2[:, :, i])
                    nc.vector.tensor_mul(s_c[i][:, :], cbp[:, 1:2], gamma_sb[:, i : i + 1])
                    nc.vector.tensor_mul(b_c[i][:, :], cbp[:, 0:1], s_c[i][:, :])
                    nc.vector.tensor_sub(b_c[i][:, :], beta_sb[:, i : i + 1], b_c[i][:, :])
                    for w_sb, wb in ((wq_sb, wqb), (wk_sb, wkb), (wv_sb, wvb)):
                        nc.vector.tensor_scalar(
                            out=wb[i][:, :],
                            in0=w_sb[i][:, :],
                            scalar1=s_c[i][:, :],
                            scalar2=None,
                            op0=ALU.mult,
                        )

                # bias vectors: qb = wq^T b (per-d, [D,1]); vb = b^T wv ([1,D]).
                # The K-side bias is dropped: its score contribution q.kb is
                # constant over the softmax (key) axis, so softmax(q.(k+kb))
                # == softmax(q.k) -- only the numerator/denominator scale by
                # the same per-query factor, which cancels.
                qbp = ps_pre.tile([D, 1], f32, name="qbp")
                vbp = ps_pre.tile([1, DP], f32, name="vbp")
                for i in range(CT):
                    nc.tensor.matmul(
                        qbp[:, :], wq_sb[i][:, :], b_c[i][:, :],
                        start=(i == 0), stop=(i == CT - 1),
                    )
                    nc.tensor.matmul(
                        vbp[:, :], b_c[i][:, :], wv_sb[i][:, :],
                        start=(i == 0), stop=(i == CT - 1),
                    )
                nc.vector.tensor_copy(qb_sb[:, :], qbp[:, :])
                nc.vector.tensor_copy(vb_sb[:, :], vbp[:, :])
                # vb col 64 = 1.0: the rank-1 bias matmul then emits the
                # ones (denominator) column of the padded v for every token.
                nc.vector.memset(vb_sb[0:1, D : D + 1], 1.0)

            s1_cm.__exit__(None, None, None)

            # ---------------- stage 3: q_T, k_T, v ----------------------------
            # Row-packed layouts for the tile_position score matmuls. Pair j
            # couples key chunk j (tokens [128j, 128j+128)) with chunk j+16
            # (second half of the token range) -- softmax is permutation-
            # invariant over keys, and this HALF-PAIRING makes every psum ->
            # SBUF copy contiguous AND partition-aligned:
            #   qTd: partitions 0-63 hold q^T, partitions 64-127 a duplicate
            #        (from the host-duplicated weight columns).
            #   kTd [128, pair, 128]: partitions 0-63 = chunks 0-15,
            #        partitions 64-127 = chunks 16-31.
            #   vpair_all [128, parity, pair, DP] fp8 (parity-major).
            kTd = pqk.tile([128, NTK // 2, TKC], bf16, name="kTd")
            qTd = pqk.tile([128, N], bf16, name="qTd")
            vpair_all = pva.tile([128, 2, NTK // 2, DP], f8, name="vpair")
            with (
                tc.tile_pool(name="ps_qk", bufs=2, space="PSUM") as ps_qk,
                tc.tile_pool(name="ps_v", bufs=2, space="PSUM") as ps_v,
            ):
                # paired psum tiles: one Identity+bias copy per 1024 tokens
                # (halves the copy instruction count; overhead-dominated on HW)
                qk_cm = scope_cm(tc, "qk")
                qk_cm.__enter__()
                for j2 in range(N // 1024):
                    pps = ps_qk.tile([D, 2, 512], f32, name="qps", tag="qkps")
                    for half in range(2):
                        j = 2 * j2 + half
                        for i in range(CT):
                            nc.tensor.matmul(
                                pps[:, half, :],
                                wqb[i][:, :],
                                xb[i][:, j * 512 : (j + 1) * 512],
                                start=(i == 0),
                                stop=(i == CT - 1),
                            )
                    nc.scalar.activation(
                        qTd[0:D, j2 * 1024 : (j2 + 1) * 1024], pps[:, :, :],
                        AF.Identity, bias=qb_sb[:, :],
                    )
                # duplicate q^T into partitions 64-127: one big contiguous
                # cross-partition DVE copy (bias already applied).
                nc.vector.tensor_copy(qTd[D : 2 * D, :], qTd[0:D, :])
                # k^T: no bias needed (see stage-2 comment). With half-
                # pairing, token groups 0-1 (chunks 0-15) fill kTd partitions
                # 0-63 and groups 2-3 (chunks 16-31) fill partitions 64-127;
                # all four are contiguous [64, 1024] copies (the latter two
                # shift partition base, which costs ~1.4x -- measured).
                for j2 in range(N // 1024):
                    pps = ps_qk.tile([D, 2, 512], f32, name="kps", tag="qkps")
                    for half in range(2):
                        j = 2 * j2 + half
                        for i in range(CT):
                            nc.tensor.matmul(
                                pps[:, half, :],
                                wkb[i][:, :],
                                xb[i][:, j * 512 : (j + 1) * 512],
                                start=(i == 0),
                                stop=(i == CT - 1),
                            )
                    base = 0 if j2 < 2 else D
                    slot = 8 * (j2 % 2)
                    if j2 % 2 == 0:
                        nc.scalar.activation(
                            kTd[base : base + D, slot : slot + 8, :],
                            pps[:, :, :],
                            AF.Copy,
                        )
                    else:
                        nc.vector.tensor_copy(
                            kTd[base : base + D, slot : slot + 8, :],
                            pps[:, :, :],
                        )
                qk_cm.__exit__(None, None, None)
                # v in [token, d] layout, fp8, already DP-padded (the zero
                # weight columns yield the pad; the rank-1 ones_tok x vb
                # matmul adds the folded GN bias AND writes the ones column
                # via vb[64] = 1). 4 token-chunks accumulate per one-bank
                # psum tile (4 x 320B = 1280B -- no matmul output crosses a
                # bank boundary) so one contiguous fp8 copy covers 4 slots.
                v_cm = scope_cm(tc, "v")
                v_cm.__enter__()
                for c4 in range(NTK // 4):
                    vps = ps_v.tile([128, 4, DP], f32, name="vps", tag="vps")
                    for sub in range(4):
                        jj = 4 * c4 + sub
                        for i in range(CT):
                            nc.tensor.matmul(
                                vps[:, sub, :],
                                xb[i][:, jj * 128 : (jj + 1) * 128],
                                wvb[i][:, :],
                                start=(i == 0),
                                stop=False,
                            )
                        nc.tensor.matmul(
                            vps[:, sub, :],
                            ones_tok[:, :],
                            vb_sb[:, :],
                            start=False,
                            stop=True,
                        )
                    nc.vector.tensor_copy(
                        vpair_all[:, c4 // 4, 4 * (c4 % 4) : 4 * (c4 % 4) + 4, :],
                        vps[:, :, :],
                    )
                v_cm.__exit__(None, None, None)

            pre_cm.__exit__(None, None, None)

            # ---------------- stage 4: attention ------------------------------
            a2a_in = pdram.tile([HEADS, D + 1, NT], bf16, name="a2a_in")
            with (
                tc.tile_pool(name="ps_s", bufs=3, space="PSUM") as ps_s,
                tc.tile_pool(name="ps_o", bufs=2, space="PSUM") as ps_o,
            ):
                import contextlib

                loop_cm = (
                    tc.For_i(
                        0,
                        attn_loop_k,
                        1,
                        hint_engines=(
                            mybir.EngineType.PE,
                            mybir.EngineType.Activation,
                        ),
                    )
                    if attn_loop_k
                    else contextlib.nullcontext()
                )
                with loop_cm:
                    _attention_stage(
                        nc, tc, ps_s, ps_o, pp, kTd, qTd, vpair_all, a2a_in
                    )

            # ---------------- stage 5: AllToAll -------------------------------
            wo_sb = [ppost.tile([128, C], f32, name=f"wo{i}") for i in range(4)]
            wob = [ppost.tile([128, C], bf16, name=f"wob{i}") for i in range(4)]
            resid_sb = [ppost.tile([128, NT], f32, name=f"res{i}") for i in range(CT)]
            bo_sb = ppost.tile([128, CT], f32, name="bo_sb")
            for i in range(4):
                nc.sync.dma_start(wo_sb[i][:, :], woT_d[i * 128 : (i + 1) * 128, :])
                nc.vector.tensor_copy(wob[i][:, :], wo_sb[i][:, :])
                nc.sync.dma_start(resid_sb[i][:, :], resid_d[i * 128 : (i + 1) * 128, :])
                nc.sync.dma_start(
                    bo_sb[:, i : i + 1], bo_d[i * 128 : (i + 1) * 128, :]
                )

            a2a_out = pdram.tile([HEADS, D + 1, NT], bf16, name="a2a_out")
            tail_cm = (
                tc.For_i(0, tail_loop_k, 1) if tail_loop_k else contextlib.nullcontext()
            )
            tail_cm.__enter__()
            nc.gpsimd.collective_compute(
                "AllToAll",
                ALU.bypass,
                replica_groups=[list(range(NCORE))],
                ins=[a2a_in.opt()],
                outs=[a2a_out.opt()],
            )

            # ---------------- stage 6: normalize + output projection ----------
            den = ppost.tile([HEADS, NT], bf16, name="den")
            nc.sync.dma_start(
                den[:, :],
                a2a_out[:, D, :],
            )
            dln = ppost.tile([HEADS, NT], f32, name="dln")
            nc.scalar.activation(dln[:, :], den[:, :], AF.Ln)
            drc = ppost.tile([HEADS, NT], f32, name="drc")
            nc.scalar.activation(drc[:, :], dln[:, :], AF.Exp, scale=-1.0)
            drc_dram = pdram.tile([HEADS, NT], f32, name="drc_dram")
            nc.sync.dma_start(drc_dram[:, :], drc[:, :])

            # batched gathers: all heads' unnormalized outputs in one DMA,
            # all heads' reciprocal denominators in one broadcast DMA
            rcv_all = ppost.tile([D, HEADS, NT], bf16, name="rcv_all")
            nc.sync.dma_start(
                rcv_all[:, :, :], a2a_out[:, 0:D, :].rearrange("h d t -> d h t")
            )
            bcr_all = ppost.tile([D, HEADS, NT], f32, name="bcr_all")
            nc.sync.dma_start(
                bcr_all[:, :, :],
                drc_dram.rearrange("h (o t) -> o (h t)", o=1).broadcast_to(
                    [D, HEADS * NT]
                ),
            )

            rhs_sb = [ppost.tile([128, NT], bf16, name=f"rhs{i}") for i in range(4)]
            for h in range(HEADS):
                nc.vector.tensor_mul(
                    rhs_sb[h // 2][(h % 2) * D : (h % 2) * D + D, :],
                    rcv_all[:, h, :],
                    bcr_all[:, h, :],
                )

            with tc.tile_pool(name="ps_y", bufs=1, space="PSUM") as ps_y:
                yps = [
                    ps_y.tile([128, NT], f32, name=f"yps{c}", tag=f"yps{c}")
                    for c in range(CT)
                ]
                for i in range(4):
                    for c in range(CT):
                        nc.tensor.matmul(
                            yps[c][:, :],
                            wob[i][:, c * 128 : (c + 1) * 128],
                            rhs_sb[i][:, :],
                            start=(i == 0),
                            stop=(i == 3),
                        )
                for c in range(CT):
                    y_sb = ppost.tile([128, NT], f32, name="y_sb", tag="y_sb", bufs=2)
                    nc.scalar.activation(
                        y_sb[:, :], yps[c][:, :], AF.Identity, bias=bo_sb[:, c : c + 1]
                    )
                    nc.vector.tensor_add(y_sb[:, :], y_sb[:, :], resid_sb[c][:, :])
                    nc.sync.dma_start(out_d[c * 128 : (c + 1) * 128, :], y_sb[:, :])

            tail_cm.__exit__(None, None, None)

    nc.compile()
    return nc


def get_nc():
    if "nc" not in _nc_cache:
        _nc_cache["nc"] = _build()
    return _nc_cache["nc"]


def make_in_maps(hidden_states, gn_gamma, gn_beta, wq, wk, wv, wo, bo):
    x2d = np.ascontiguousarray(
        np.asarray(hidden_states, dtype=np.float32).reshape(C, N)
    )
    gamma = np.ascontiguousarray(np.asarray(gn_gamma, np.float32).reshape(C, 1))
    beta = np.ascontiguousarray(np.asarray(gn_beta, np.float32).reshape(C, 1))
    wq = np.asarray(wq, np.float32)
    wk = np.asarray(wk, np.float32)
    wv = np.asarray(wv, np.float32)
    woT = np.ascontiguousarray(np.asarray(wo, np.float32).T)
    bo2 = np.ascontiguousarray(np.asarray(bo, np.float32).reshape(C, 1))
    bones = np.zeros((128, 8), np.float32)
    for cc in range(128):
        bones[cc, cc // CPG] = 1.0
    bonesT = np.ascontiguousarray(bones.T)
    in_maps = []
    for h in range(NCORE):
        sl = slice(h * D, (h + 1) * D)
        in_maps.append(
            {
                "x": x2d,
                "gamma": gamma,
                "beta": beta,
                "wqT": np.ascontiguousarray(wq[sl, :].T),
                "wkT": np.ascontiguousarray(wk[sl, :].T),
                "wvT": np.ascontiguousarray(
                    np.concatenate(
                        [wv[sl, :].T, np.zeros((C, DP - D), np.float32)], axis=1
                    )
                ),
                "woT": woT,
                "bo": bo2,
                "resid": np.ascontiguousarray(x2d[:, h * NT : (h + 1) * NT]),
                "bones": bones,
                "bonesT": bonesT,
            }
        )
    return in_maps


def kernel(hidden_states, gn_gamma, gn_beta, wq, wk, wv, wo, bo):
    nc = get_nc()
    in_maps = make_in_maps(hidden_states, gn_gamma, gn_beta, wq, wk, wv, wo, bo)
    res = run_bass_kernel_spmd(nc, in_maps, core_ids=list(range(NCORE)))
    out2d = np.empty((C, N), np.float32)
    for h in range(NCORE):
        out2d[:, h * NT : (h + 1) * NT] = res.results[h]["out"]
    return out2d.reshape(1, C, 64, 64)



# revision 57
# speedup vs baseline: 1.1621x; 1.1621x over previous
"""Trainium2 Bass kernel for GroupNorm + spatial self-attention + residual.

Reference computation (B=1, C=512, H=W=64, 8 heads x 64 dim, GN groups=32):
    x = GroupNorm(hidden_states) -> tokens [N=4096, C]
    q,k,v = x @ {wq,wk,wv}.T  (per-head slices of inner=512)
    out = softmax(q k^T / 8) v   per head
    y = concat_heads(out) @ wo.T + bo + hidden_states

Distribution over 8 NeuronCores: head-parallel attention (core h owns head h;
every core reads the full input), then an AllToAll that token-shards the
attention output so core j computes the output projection + bias + residual
for tokens [512j, 512j+512) only.

Per-core device graph (SPMD, same graph on all 8 cores, per-core data differs):
  1. DMA x [512, 4096] f32 into SBUF; xb = raw bf16 cast on ScalarE (no
     affine) overlapped with per-channel stats on VectorE; per-tile group
     stats via block-ones matmuls; rstd = exp(-0.5 ln(var+eps)).
  2. The GroupNorm affine is FOLDED INTO THE PROJECTION WEIGHTS:
     w' = w * s per channel (cast bf16), plus rank-1 bias terms
     qb = wq^T b (applied in the psum->sbuf copy via Identity+bias) and
     vb = b^T wv (applied as an extra K=1 matmul in the v accumulation).
     The K-side bias is DROPPED: its score term q.kb is constant along the
     softmax (key) axis, so softmax is unchanged.
  3. Row-packed layouts for tile_position score matmuls: qTd [128, 4096]
     holds q^T duplicated in both partition halves; kTd [128, 16, 128]
     holds key chunks 0-15 (tokens 0-2047) in partitions 0-63 and chunks
     16-31 in partitions 64-127 (pair j = chunks (j, j+16); softmax is
     permutation-invariant over keys, and this HALF-PAIRING keeps every
     psum->SBUF copy contiguous). v in fp8e4 [128, parity, pair, 80]:
     cols 0-63 = v (GN bias included), col 64 = ones -- emitted by the
     host-zero-padded wv plus vb[64]=1 through the rank-1 bias matmul.
  4. Flash-style attention in transposed layout: the two 128-key chunks of
     a pair run CONCURRENTLY as 64-row tile_position row-tiles (K=64 only
     fills half the PE array; packing doubles score throughput) into the
     two banks of a pair-sized psum tile; ONE exp instruction per 256-key
     pair (alternating ScalarE Exp / VectorE EXP16, fp8 out); PV runs fp8
     DoubleRow with the 80-col padded stationary so rows 0-63 accumulate
     PV and row 64 accumulates the softmax denominator IN THE SAME MATMUL
     (the 80-col pad satisfies the dual-fp8 ldweights 16B step rule).
     No max-subtraction (scores are O(1) by construction).
  5. Unnormalized out [65, 512] per query block (row 64 = denominators),
     one ScalarE copy, bf16 -> DRAM -> AllToAll.
  6. Each core normalizes its received token chunk (ln/exp reciprocal,
     batched gather + broadcast DMAs), output projection (i-outer matmuls),
     +bo, +residual, writes its [512, 512] column chunk of the output.
"""

import sys

sys.path.insert(0, "/opt/trn_rl_repo")

import numpy as np

import concourse.bacc as bacc
import concourse.tile as tile
from concourse import mybir
from concourse.bass_utils import run_bass_kernel_spmd

C = 512
N = 4096
HEADS = 8
D = 64
GROUPS = 32
CPG = C // GROUPS  # 16 channels per group
EPS = 1e-5
SCALE = D ** -0.5
NCORE = 8
NT = N // NCORE  # 512 tokens per core for the output projection
TQ = 512  # query-chunk (free dim of transposed scores); one psum bank
NTQ = N // TQ  # 8
TKC = 128  # key-chunk (partition dim of transposed scores)
NTK = N // TKC  # 32
CT = C // 128  # 4 channel tiles
DP = 80  # fp8 V stationary padded to 80 cols: col 64 = ones (denominator),
#          cols 65-79 = zero pad so the DoubleRow weights step is 16B-aligned

f32 = mybir.dt.float32
bf16 = mybir.dt.bfloat16
f8 = mybir.dt.float8e4
AF = mybir.ActivationFunctionType
ALU = mybir.AluOpType

_nc_cache = {}

# exp(SCALE*x) ~= ((x*EC0 + EC1)^2 + 0.5)^16  -- a (1 + y/16 + y^2/512)^16
# approximation computed in one fused VectorE pass (8 ALU stages), used to
# split softmax exp work between ScalarE and VectorE. Max rel err 2.9e-3 at
# |y|=1.6 (scores here stay well inside that), 3.5e-4 for |y|<0.8.
EC0 = SCALE / float(np.sqrt(512.0))
EC1 = float(np.sqrt(0.5))
SKEW_PAIRS = 3  # PV consumes pair j while scores/exp work on pair j+SKEW

# ACT/DVE exp split, one entry per 256-key PAIR: measured HW per-pair costs
# are near-equal (ScalarE ~1.68us vs VectorE-EXP16 ~1.72us incl. overhead),
# so strict alternation balances load and keeps consecutive pairs on
# different engines for maximum overlap.
EXP_DVE_PAT = tuple(j % 2 == 1 for j in range(NTK))


def _register_exp16():
    from concourse import dve_ops as dops
    from concourse.dve_spec import Spec, Src0, C0, C1, sq

    for op in dops.OPS:
        if op.name == "EXP16_ANT":
            return op
    t = sq(Src0 * C0 + C1) + C2_LEAF
    body = sq(sq(sq(sq(t))))
    spec = Spec(
        body=body,
        reference=lambda in0, in1, s0, s1, imm2: ((in0 * s0 + s1) ** 2 + imm2)
        ** 16,
    )
    op = dops.DveOp("EXP16_ANT", spec, subdim=False, uops_sha={})
    dops.OPS.append(op)
    dops.CUSTOM_DVE_SPECS[op.name] = op.spec
    dops._SUB_OPCODE_FOR_NAME[op.name] = dops._CUSTOM_DVE_ROW_BASE + len(dops.OPS) - 1
    from concourse.dve_uop import DveOpSpec
    from concourse.dve_spec import lower as dve_lower

    for ver in ("v3", "v4"):
        try:
            uops = dve_lower(spec, ver=ver)
            sha = DveOpSpec(
                name=op.name,
                opcode=dops.get_dve_sub_opcode(op.name),
                uops=uops,
                rd1_en=False,
            ).sha(ver)
            op.uops_sha[ver] = sha
        except Exception:
            pass
    return op


from concourse.dve_spec import C2 as C2_LEAF  # noqa: E402

EXP16 = _register_exp16()


def _attention_stage(nc, tc, ps_s, ps_o, pp, kTd, qTd, vpair_all, a2a_in):
    NPAIR = NTK // 2  # 16 pairs of 128-key chunks; PV runs fp8 DoubleRow
    PM = mybir.MatmulPerfMode.DoubleRow
    for jq in range(NTQ):
        # merged PV+den DoubleRow output: rows 0-63 = PV, row 64 = the ones
        # column denominator, rows 65-79 = padding (the fp8 weights are
        # padded to 80 cols so the dual-fp8 ldweights step%16==0 rule holds).
        ops = ps_o.tile([DP, TQ], f32, name="ops", tag="ops")
        p_tiles = {}

        sps_tiles = {}

        def mm_scores(tk):
            # pair-sized score tile: each 512-col matmul lands in one psum
            # bank; the pair is exp'd by a SINGLE instruction (per-instruction
            # overhead on HW is ~800ns, so fewer/bigger exps win).
            # K=64 contraction uses half the PE rows, so the even chunk runs
            # in row-group 0-1 (partitions 0-63) and the odd chunk runs
            # CONCURRENTLY in row-group 2-3 (partitions 64-127) via
            # tile_position row packing -- ~2x score throughput.
            j, half = tk // 2, tk % 2
            if half == 0:
                sps_tiles[j] = ps_s.tile(
                    [128, 2, TQ], f32, name="sps", tag="sps", bufs=3
                )
            sps = sps_tiles[j]
            base = 64 * half
            nc.tensor.matmul(
                sps[:, half, :],
                kTd[base : base + 64, j, :],
                qTd[base : base + 64, jq * TQ : (jq + 1) * TQ],
                start=True,
                stop=True,
            )
            if half == 0:
                return
            p_tiles[j] = pp.tile([128, 2, TQ], f8, name="p", tag="p", bufs=6)
            p = p_tiles[j]
            sps = sps_tiles.pop(j)
            if EXP_DVE_PAT[j % len(EXP_DVE_PAT)]:
                nc.vector._custom_dve(
                    EXP16, out=p[:, :, :], in0=sps[:, :, :],
                    s0=EC0, s1=EC1, imm2=0.5,
                )
            else:
                nc.scalar.activation(
                    p[:, :, :], sps[:, :, :], AF.Exp, scale=SCALE
                )

        def mm_pvden(j):
            p = p_tiles.pop(j)
            nc.tensor.matmul(
                ops[:, :],
                vpair_all[:, :, j, :],
                p[:, :, :],
                start=(j == 0),
                stop=(j == NPAIR - 1),
                perf_mode=PM,
            )

        # software pipeline: scores/exp run SKEW_PAIRS pair-steps ahead of
        # the PV accumulation so the PE never waits on a fresh exp.
        for tk in range(2 * SKEW_PAIRS):
            mm_scores(tk)
        for j in range(SKEW_PAIRS, NPAIR):
            mm_scores(2 * j)
            mm_scores(2 * j + 1)
            mm_pvden(j - SKEW_PAIRS)
        for j in range(NPAIR - SKEW_PAIRS, NPAIR):
            mm_pvden(j)

        # always ACT: the exp split leaves ACT with slack while DVE binds
        o_sb = pp.tile([D + 1, TQ], bf16, name="o_sb", tag="o_sb", bufs=2)
        nc.scalar.activation(o_sb[:, :], ops[0 : D + 1, :], AF.Copy)
        nc.sync.dma_start(a2a_in[jq, :, :], o_sb[:, :])


def _build(attn_loop_k=None, pre_loop_k=None, tail_loop_k=None, pre_scope="all"):
    import contextlib
    nc = bacc.Bacc("TRN2", target_bir_lowering=False, debug=False, num_devices=NCORE)

    def scope_cm(tc, name):
        # For_i around just one pre sub-stage when pre_scope selects it
        if pre_loop_k and pre_scope == name:
            return tc.For_i(0, pre_loop_k, 1)
        return contextlib.nullcontext()

    x_d = nc.dram_tensor("x", [C, N], f32, kind="ExternalInput")
    gamma_d = nc.dram_tensor("gamma", [C, 1], f32, kind="ExternalInput")
    beta_d = nc.dram_tensor("beta", [C, 1], f32, kind="ExternalInput")
    wqT_d = nc.dram_tensor("wqT", [C, D], f32, kind="ExternalInput")
    wkT_d = nc.dram_tensor("wkT", [C, D], f32, kind="ExternalInput")
    # wv arrives zero-padded to [C, DP]: the v matmuls then emit the padded
    # 80-col layout directly and the ones (denominator) column comes from
    # the rank-1 bias matmul with vb[64] overwritten to 1.0.
    wvT_d = nc.dram_tensor("wvT", [C, DP], f32, kind="ExternalInput")
    woT_d = nc.dram_tensor("woT", [C, C], f32, kind="ExternalInput")
    bo_d = nc.dram_tensor("bo", [C, 1], f32, kind="ExternalInput")
    resid_d = nc.dram_tensor("resid", [C, NT], f32, kind="ExternalInput")
    bones_d = nc.dram_tensor("bones", [128, 8], f32, kind="ExternalInput")
    bonesT_d = nc.dram_tensor("bonesT", [8, 128], f32, kind="ExternalInput")
    out_d = nc.dram_tensor("out", [C, NT], f32, kind="ExternalOutput")

    with tile.TileContext(nc) as tc:
        with (
            tc.tile_pool(name="xc", bufs=1) as pxc,
            tc.tile_pool(name="xb", bufs=1) as pxb,
            tc.tile_pool(name="qk", bufs=1) as pqk,
            tc.tile_pool(name="vaug", bufs=1) as pva,
            tc.tile_pool(name="w", bufs=1) as pw,
            tc.tile_pool(name="small", bufs=1) as psm,
            tc.tile_pool(name="p", bufs=3) as pp,
            tc.tile_pool(name="post", bufs=1) as ppost,
            tc.tile_pool(name="dram", bufs=1, space="DRAM") as pdram,
        ):
            # ---------------- stage 1: load x + GroupNorm statistics ----------
            pre_cm = (
                tc.For_i(0, pre_loop_k, 1)
                if pre_loop_k and pre_scope == "all"
                else contextlib.nullcontext()
            )
            pre_cm.__enter__()
            s1_cm = scope_cm(tc, "s1")
            s1_cm.__enter__()
            # ---- x DMAs first (the long pole), params after ----------------
            xc = [pxc.tile([128, N], f32, name=f"xc{i}") for i in range(CT)]
            for i in range(CT):
                nc.sync.dma_start(xc[i][:, :], x_d[i * 128 : (i + 1) * 128, :])
            bones = psm.tile([128, 8], f32, name="bones")
            bonesT = psm.tile([8, 128], f32, name="bonesT")
            gamma_sb = psm.tile([128, CT], f32, name="gamma_sb")
            beta_sb = psm.tile([128, CT], f32, name="beta_sb")
            nc.sync.dma_start(bones[:, :], bones_d[:, :])
            nc.sync.dma_start(bonesT[:, :], bonesT_d[:, :])
            nc.sync.dma_start(
                gamma_sb[:, :], gamma_d.rearrange("(t p) o -> p (t o)", p=128)
            )
            nc.sync.dma_start(
                beta_sb[:, :], beta_d.rearrange("(t p) o -> p (t o)", p=128)
            )
            wq_sb = [pw.tile([128, D], f32, name=f"wq{i}") for i in range(CT)]
            wk_sb = [pw.tile([128, D], f32, name=f"wk{i}") for i in range(CT)]
            wv_sb = [pw.tile([128, DP], f32, name=f"wv{i}") for i in range(CT)]
            for i in range(CT):
                nc.sync.dma_start(wq_sb[i][:, :], wqT_d[i * 128 : (i + 1) * 128, :])
                nc.sync.dma_start(wk_sb[i][:, :], wkT_d[i * 128 : (i + 1) * 128, :])
                nc.sync.dma_start(wv_sb[i][:, :], wvT_d[i * 128 : (i + 1) * 128, :])

            # ---- per tile: raw bf16 cast (ACT) + channel stats (DVE) +
            # ---- per-tile group sums (PE): GN affine is folded into the
            # ---- projection weights later, so the cast needs no stats.
            xb = [pxb.tile([128, N], bf16, name=f"xb{i}") for i in range(CT)]
            cstat = [psm.tile([128, 2], f32, name=f"cs{i}") for i in range(CT)]
            gstat = psm.tile([8, 2, CT], f32, name="gstat")
            s_c = [psm.tile([128, 1], f32, name=f"s_c{i}") for i in range(CT)]
            b_c = [psm.tile([128, 1], f32, name=f"b_c{i}") for i in range(CT)]
            wqb = [pw.tile([128, D], bf16, name=f"wqb{i}") for i in range(CT)]
            wkb = [pw.tile([128, D], bf16, name=f"wkb{i}") for i in range(CT)]
            wvb = [pw.tile([128, DP], bf16, name=f"wvb{i}") for i in range(CT)]
            qb_sb = psm.tile([D, 1], f32, name="qb_sb")
            vb_sb = psm.tile([1, DP], bf16, name="vb_sb")
            ones_tok = psm.tile([1, 128], bf16, name="ones_tok")
            nc.vector.memset(ones_tok[:, :], 1.0)

            with tc.tile_pool(name="ps_pre", bufs=1, space="PSUM") as ps_pre:
                for i in range(CT):
                    # channel sums ride the bf16 cast for free (ACT accum);
                    # channel sum-of-squares is ONE big DVE pass per tile
                    # (bn_stats would need 8 instructions at the 512-free cap,
                    # and per-instruction overhead dominates on HW).
                    sx = psm.tile([128, 1], f32, name="sx", tag="sx", bufs=2)
                    nc.scalar.activation(
                        xb[i][:, :], xc[i][:, :], AF.Copy, accum_out=sx[:, :]
                    )
                    sq_scr = psm.tile([128, N], bf16, name="sq_scr", tag="sq", bufs=1)
                    sxx = psm.tile([128, 1], f32, name="sxx", tag="sxx", bufs=2)
                    nc.vector.scalar_tensor_tensor(
                        out=sq_scr[:, :], in0=xc[i][:, :], scalar=1.0,
                        in1=xc[i][:, :], op0=ALU.mult, op1=ALU.mult,
                        accum_out=sxx[:, :],
                    )
                    # cstat = (mean, E[x^2]) per channel
                    nc.vector.tensor_scalar_mul(cstat[i][:, 0:1], sx[:, :], 1.0 / N)
                    nc.vector.tensor_scalar_mul(cstat[i][:, 1:2], sxx[:, :], 1.0 / N)
                    gps = ps_pre.tile([8, 2], f32, name="gps", tag="gps", bufs=2)
                    nc.tensor.matmul(gps[:, :], bones[:, :], cstat[i][:, :])
                    nc.vector.tensor_copy(gstat[:, :, i], gps[:, :])

                # group mean/ex2 -> mean, rstd (layout [group8, stat2, tile4])
                gm = psm.tile([8, 2, CT], f32, name="gm")
                nc.vector.tensor_scalar_mul(gm[:, :, :], gstat[:, :, :], 1.0 / CPG)
                vtmp = psm.tile([8, 1, CT], f32, name="vtmp")
                nc.vector.tensor_mul(vtmp[:, 0, :], gm[:, 0, :], gm[:, 0, :])
                varg = psm.tile([8, 1, CT], f32, name="varg")
                nc.vector.tensor_sub(varg[:, 0, :], gm[:, 1, :], vtmp[:, 0, :])
                eps_sb = psm.tile([8, 1], f32, name="eps_sb")
                nc.vector.memset(eps_sb[:, :], EPS)
                lng = psm.tile([8, 1, CT], f32, name="lng")
                nc.scalar.activation(lng[:, 0, :], varg[:, 0, :], AF.Ln, bias=eps_sb[:, :])
                rstd = psm.tile([8, 1, CT], f32, name="rstd")
                nc.scalar.activation(rstd[:, 0, :], lng[:, 0, :], AF.Exp, scale=-0.5)
                gs2 = psm.tile([8, 2, CT], f32, name="gs2")
                nc.vector.tensor_copy(gs2[:, 0, :], gm[:, 0, :])
                nc.vector.tensor_copy(gs2[:, 1, :], rstd[:, 0, :])

                # expand group (mean, rstd) -> per-channel via bonesT matmul;
                # fold the GN affine into the projection weights
                for i in range(CT):
                    cbp = ps_pre.tile([128, 2], f32, name="cbp", tag="cbp", bufs=2)
                    nc.tensor.matmul(cbp[:, :], bonesT[:, :], gs2[:, :, i])
                    nc.vector.tensor_mul(s_c[i][:, :], cbp[:, 1:2], gamma_sb[:, i : i + 1])
                    nc.vector.tensor_mul(b_c[i][:, :], cbp[:, 0:1], s_c[i][:, :])
                    nc.vector.tensor_sub(b_c[i][:, :], beta_sb[:, i : i + 1], b_c[i][:, :])
                    for w_sb, wb in ((wq_sb, wqb), (wk_sb, wkb), (wv_sb, wvb)):
                        nc.vector.tensor_scalar(
                            out=wb[i][:, :],
                            in0=w_sb[i][:, :],
                            scalar1=s_c[i][:, :],
                            scalar2=None,
                            op0=ALU.mult,
                        )

                # bias vectors: qb = wq^T b (per-d, [D,1]); vb = b^T wv ([1,D]).
                # The K-side bias is dropped: its score contribution q.kb is
                # constant over the softmax (key) axis, so softmax(q.(k+kb))
                # == softmax(q.k) -- only the numerator/denominator scale by
                # the same per-query factor, which cancels.
                qbp = ps_pre.tile([D, 1], f32, name="qbp")
                vbp = ps_pre.tile([1, DP], f32, name="vbp")
                for i in range(CT):
                    nc.tensor.matmul(
                        qbp[:, :], wq_sb[i][:, :], b_c[i][:, :],
                        start=(i == 0), stop=(i == CT - 1),
                    )
                    nc.tensor.matmul(
                        vbp[:, :], b_c[i][:, :], wv_sb[i][:, :],
                        start=(i == 0), stop=(i == CT - 1),
                    )
                nc.vector.tensor_copy(qb_sb[:, :], qbp[:, :])
                nc.vector.tensor_copy(vb_sb[:, :], vbp[:, :])
                # vb col 64 = 1.0: the rank-1 bias matmul then emits the
                # ones (denominator) column of the padded v for every token.
                nc.vector.memset(vb_sb[0:1, D : D + 1], 1.0)

            s1_cm.__exit__(None, None, None)

            # ---------------- stage 3: q_T, k_T, v ----------------------------
            # Row-packed layouts for the tile_position score matmuls. Pair j
            # couples key chunk j (tokens [128j, 128j+128)) with chunk j+16
            # (second half of the token range) -- softmax is permutation-
            # invariant over keys, and this HALF-PAIRING makes every psum ->
            # SBUF copy contiguous AND partition-aligned:
            #   qTd: partitions 0-63 hold q^T, partitions 64-127 a duplicate
            #        (from the host-duplicated weight columns).
            #   kTd [128, pair, 128]: partitions 0-63 = chunks 0-15,
            #        partitions 64-127 = chunks 16-31.
            #   vpair_all [128, parity, pair, DP] fp8 (parity-major).
            kTd = pqk.tile([128, NTK // 2, TKC], bf16, name="kTd")
            qTd = pqk.tile([128, N], bf16, name="qTd")
            vpair_all = pva.tile([128, 2, NTK // 2, DP], f8, name="vpair")
            with (
                tc.tile_pool(name="ps_qk", bufs=2, space="PSUM") as ps_qk,
                tc.tile_pool(name="ps_v", bufs=2, space="PSUM") as ps_v,
            ):
                # paired psum tiles: one Identity+bias copy per 1024 tokens
                # (halves the copy instruction count; overhead-dominated on HW)
                qk_cm = scope_cm(tc, "qk")
                qk_cm.__enter__()
                for j2 in range(N // 1024):
                    pps = ps_qk.tile([D, 2, 512], f32, name="qps", tag="qkps")
                    for half in range(2):
                        j = 2 * j2 + half
                        for i in range(CT):
                            nc.tensor.matmul(
                                pps[:, half, :],
                                wqb[i][:, :],
                                xb[i][:, j * 512 : (j + 1) * 512],
                                start=(i == 0),
                                stop=(i == CT - 1),
                            )
                    nc.scalar.activation(
                        qTd[0:D, j2 * 1024 : (j2 + 1) * 1024], pps[:, :, :],
                        AF.Identity, bias=qb_sb[:, :],
                    )
                # duplicate q^T into partitions 64-127: one big contiguous
                # cross-partition DVE copy (bias already applied).
                nc.vector.tensor_copy(qTd[D : 2 * D, :], qTd[0:D, :])
                # k^T: no bias needed (see stage-2 comment). With half-
                # pairing, token groups 0-1 (chunks 0-15) fill kTd partitions
                # 0-63 and groups 2-3 (chunks 16-31) fill partitions 64-127;
                # all four are contiguous [64, 1024] copies (the latter two
                # shift partition base, which costs ~1.4x -- measured).
                for j2 in range(N // 1024):
                    pps = ps_qk.tile([D, 2, 512], f32, name="kps", tag="qkps")
                    for half in range(2):
                        j = 2 * j2 + half
                        for i in range(CT):
                            nc.tensor.matmul(
                                pps[:, half, :],
                                wkb[i][:, :],
                                xb[i][:, j * 512 : (j + 1) * 512],
                                start=(i == 0),
                                stop=(i == CT - 1),
                            )
                    base = 0 if j2 < 2 else D
                    slot = 8 * (j2 % 2)
                    if j2 % 2 == 0:
                        nc.scalar.activation(
                            kTd[base : base + D, slot : slot + 8, :],
                            pps[:, :, :],
                            AF.Copy,
                        )
                    else:
                        nc.vector.tensor_copy(
                            kTd[base : base + D, slot : slot + 8, :],
                            pps[:, :, :],
                        )
                qk_cm.__exit__(None, None, None)
                # v in [token, d] layout, fp8, already DP-padded (the zero
                # weight columns yield the pad; the rank-1 ones_tok x vb
                # matmul adds the folded GN bias AND writes the ones column
                # via vb[64] = 1). 4 token-chunks accumulate per one-bank
                # psum tile (4 x 320B = 1280B -- no matmul output crosses a
                # bank boundary) so one contiguous fp8 copy covers 4 slots.
                v_cm = scope_cm(tc, "v")
                v_cm.__enter__()
                for c4 in range(NTK // 4):
                    vps = ps_v.tile([128, 4, DP], f32, name="vps", tag="vps")
                    for sub in range(4):
                        jj = 4 * c4 + sub
                        for i in range(CT):
                            nc.tensor.matmul(
                                vps[:, sub, :],
                                xb[i][:, jj * 128 : (jj + 1) * 128],
                                wvb[i][:, :],
                                start=(i == 0),
                                stop=False,
                            )
                        nc.tensor.matmul(
                            vps[:, sub, :],
                            ones_tok[:, :],
                            vb_sb[:, :],
                            start=False,
                            stop=True,
                        )
                    nc.vector.tensor_copy(
                        vpair_all[:, c4 // 4, 4 * (c4 % 4) : 4 * (c4 % 4) + 4, :],
                        vps[:, :, :],
                    )
                v_cm.__exit__(None, None, None)

            pre_cm.__exit__(None, None, None)

            # ---------------- stage 4: attention ------------------------------
            a2a_in = pdram.tile([HEADS, D + 1, NT], bf16, name="a2a_in")
            with (
                tc.tile_pool(name="ps_s", bufs=3, space="PSUM") as ps_s,
                tc.tile_pool(name="ps_o", bufs=2, space="PSUM") as ps_o,
            ):
                import contextlib

                loop_cm = (
                    tc.For_i(
                        0,
                        attn_loop_k,
                        1,
                        hint_engines=(
                            mybir.EngineType.PE,
                            mybir.EngineType.Activation,
                        ),
                    )
                    if attn_loop_k
                    else contextlib.nullcontext()
                )
                with loop_cm:
                    _attention_stage(
                        nc, tc, ps_s, ps_o, pp, kTd, qTd, vpair_all, a2a_in
                    )

            # ---------------- stage 5: AllToAll -------------------------------
            wo_sb = [ppost.tile([128, C], f32, name=f"wo{i}") for i in range(4)]
            wob = [ppost.tile([128, C], bf16, name=f"wob{i}") for i in range(4)]
            resid_sb = [ppost.tile([128, NT], f32, name=f"res{i}") for i in range(CT)]
            bo_sb = ppost.tile([128, CT], f32, name="bo_sb")
            for i in range(4):
                nc.sync.dma_start(wo_sb[i][:, :], woT_d[i * 128 : (i + 1) * 128, :])
                nc.vector.tensor_copy(wob[i][:, :], wo_sb[i][:, :])
                nc.sync.dma_start(resid_sb[i][:, :], resid_d[i * 128 : (i + 1) * 128, :])
                nc.sync.dma_start(
                    bo_sb[:, i : i + 1], bo_d[i * 128 : (i + 1) * 128, :]
                )

            a2a_out = pdram.tile([HEADS, D + 1, NT], bf16, name="a2a_out")
            tail_cm = (
                tc.For_i(0, tail_loop_k, 1) if tail_loop_k else contextlib.nullcontext()
            )
            tail_cm.__enter__()
            nc.gpsimd.collective_compute(
                "AllToAll",
                ALU.bypass,
                replica_groups=[list(range(NCORE))],
                ins=[a2a_in.opt()],
                outs=[a2a_out.opt()],
            )

            # ---------------- stage 6: normalize + output projection ----------
            den = ppost.tile([HEADS, NT], bf16, name="den")
            nc.sync.dma_start(
                den[:, :],
                a2a_out[:, D, :],
            )
            dln = ppost.tile([HEADS, NT], f32, name="dln")
            nc.scalar.activation(dln[:, :], den[:, :], AF.Ln)
            drc = ppost.tile([HEADS, NT], f32, name="drc")
            nc.scalar.activation(drc[:, :], dln[:, :], AF.Exp, scale=-1.0)
            drc_dram = pdram.tile([HEADS, NT], f32, name="drc_dram")
            nc.sync.dma_start(drc_dram[:, :], drc[:, :])

            # batched gathers: all heads' unnormalized outputs in one DMA,
            # all heads' reciprocal denominators in one broadcast DMA
            rcv_all = ppost.tile([D, HEADS, NT], bf16, name="rcv_all")
            nc.sync.dma_start(
                rcv_all[:, :, :], a2a_out[:, 0:D, :].rearrange("h d t -> d h t")
            )
            bcr_all = ppost.tile([D, HEADS, NT], f32, name="bcr_all")
            nc.sync.dma_start(
                bcr_all[:, :, :],
                drc_dram.rearrange("h (o t) -> o (h t)", o=1).broadcast_to(
                    [D, HEADS * NT]
                ),
            )

            rhs_sb = [ppost.tile([128, NT], bf16, name=f"rhs{i}") for i in range(4)]
            for h in range(HEADS):
                nc.vector.tensor_mul(
                    rhs_sb[h // 2][(h % 2) * D : (h % 2) * D + D, :],
                    rcv_all[:, h, :],
                    bcr_all[:, h, :],
                )

            with tc.tile_pool(name="ps_y", bufs=1, space="PSUM") as ps_y:
                yps = [
                    ps_y.tile([128, NT], f32, name=f"yps{c}", tag=f"yps{c}")
                    for c in range(CT)
                ]
                for i in range(4):
                    for c in range(CT):
                        nc.tensor.matmul(
                            yps[c][:, :],
                            wob[i][:, c * 128 : (c + 1) * 128],
                            rhs_sb[i][:, :],
                            start=(i == 0),
                            stop=(i == 3),
                        )
                for c in range(CT):
                    y_sb = ppost.tile([128, NT], f32, name="y_sb", tag="y_sb", bufs=2)
                    nc.scalar.activation(
                        y_sb[:, :], yps[c][:, :], AF.Identity, bias=bo_sb[:, c : c + 1]
                    )
                    nc.vector.tensor_add(y_sb[:, :], y_sb[:, :], resid_sb[c][:, :])
                    nc.sync.dma_start(out_d[c * 128 : (c + 1) * 128, :], y_sb[:, :])

            tail_cm.__exit__(None, None, None)

    nc.compile()
    return nc


def get_nc():
    if "nc" not in _nc_cache:
        _nc_cache["nc"] = _build()
    return _nc_cache["nc"]


def make_in_maps(hidden_states, gn_gamma, gn_beta, wq, wk, wv, wo, bo):
    x2d = np.ascontiguousarray(
        np.asarray(hidden_states, dtype=np.float32).reshape(C, N)
    )
    gamma = np.ascontiguousarray(np.asarray(gn_gamma, np.float32).reshape(C, 1))
    beta = np.ascontiguousarray(np.asarray(gn_beta, np.float32).reshape(C, 1))
    wq = np.asarray(wq, np.float32)
    wk = np.asarray(wk, np.float32)
    wv = np.asarray(wv, np.float32)
    woT = np.ascontiguousarray(np.asarray(wo, np.float32).T)
    bo2 = np.ascontiguousarray(np.asarray(bo, np.float32).reshape(C, 1))
    bones = np.zeros((128, 8), np.float32)
    for cc in range(128):
        bones[cc, cc // CPG] = 1.0
    bonesT = np.ascontiguousarray(bones.T)
    in_maps = []
    for h in range(NCORE):
        sl = slice(h * D, (h + 1) * D)
        in_maps.append(
            {
                "x": x2d,
                "gamma": gamma,
                "beta": beta,
                "wqT": np.ascontiguousarray(wq[sl, :].T),
                "wkT": np.ascontiguousarray(wk[sl, :].T),
                "wvT": np.ascontiguousarray(
                    np.concatenate(
                        [wv[sl, :].T, np.zeros((C, DP - D), np.float32)], axis=1
                    )
                ),
                "woT": woT,
                "bo": bo2,
                "resid": np.ascontiguousarray(x2d[:, h * NT : (h + 1) * NT]),
                "bones": bones,
                "bonesT": bonesT,
            }
        )
    return in_maps


def kernel(hidden_states, gn_gamma, gn_beta, wq, wk, wv, wo, bo):
    nc = get_nc()
    in_maps = make_in_maps(hidden_states, gn_gamma, gn_beta, wq, wk, wv, wo, bo)
    res = run_bass_kernel_spmd(nc, in_maps, core_ids=list(range(NCORE)))
    out2d = np.empty((C, N), np.float32)
    for h in range(NCORE):
        out2d[:, h * NT : (h + 1) * NT] = res.results[h]["out"]
    return out2d.reshape(1, C, 64, 64)



# revision 59
# speedup vs baseline: 1.4296x; 1.2302x over previous
"""Trainium2 Bass kernel for GroupNorm + spatial self-attention + residual.

Reference computation (B=1, C=512, H=W=64, 8 heads x 64 dim, GN groups=32):
    x = GroupNorm(hidden_states) -> tokens [N=4096, C]
    q,k,v = x @ {wq,wk,wv}.T  (per-head slices of inner=512)
    out = softmax(q k^T / 8) v   per head
    y = concat_heads(out) @ wo.T + bo + hidden_states

Distribution over 8 NeuronCores: head-parallel attention (core h owns head h;
every core reads the full input), then an AllToAll that token-shards the
attention output so core j computes the output projection + bias + residual
for tokens [512j, 512j+512) only.

Per-core device graph (SPMD, same graph on all 8 cores, per-core data differs):
  1. DMA x [512, 4096] f32 into SBUF; xb = raw bf16 cast on ScalarE (no
     affine) overlapped with per-channel stats on VectorE; per-tile group
     stats via block-ones matmuls; rstd = exp(-0.5 ln(var+eps)).
  2. The GroupNorm affine is FOLDED INTO THE PROJECTION WEIGHTS:
     w' = w * s per channel (cast bf16), plus rank-1 bias terms
     qb = wq^T b (applied in the psum->sbuf copy via Identity+bias) and
     vb = b^T wv (applied as an extra K=1 matmul in the v accumulation).
     The K-side bias is DROPPED: its score term q.kb is constant along the
     softmax (key) axis, so softmax is unchanged.
  3. Row-packed layouts for tile_position score matmuls: qTd [128, 4096]
     holds q^T duplicated in both partition halves; kTd [128, 16, 128]
     holds key chunks 0-15 (tokens 0-2047) in partitions 0-63 and chunks
     16-31 in partitions 64-127 (pair j = chunks (j, j+16); softmax is
     permutation-invariant over keys, and this HALF-PAIRING keeps every
     psum->SBUF copy contiguous). v in fp8e4 [128, parity, pair, 80]:
     cols 0-63 = v (GN bias included), col 64 = ones -- emitted by the
     host-zero-padded wv plus vb[64]=1 through the rank-1 bias matmul.
  4. Flash-style attention in transposed layout: the two 128-key chunks of
     a pair run CONCURRENTLY as 64-row tile_position row-tiles (K=64 only
     fills half the PE array; packing doubles score throughput) into the
     two banks of a pair-sized psum tile; ONE exp instruction per 256-key
     pair (alternating ScalarE Exp / VectorE EXP16, fp8 out); PV runs fp8
     DoubleRow with the 80-col padded stationary so rows 0-63 accumulate
     PV and row 64 accumulates the softmax denominator IN THE SAME MATMUL
     (the 80-col pad satisfies the dual-fp8 ldweights 16B step rule).
     No max-subtraction (scores are O(1) by construction).
  5. Unnormalized out [65, 512] per query block (row 64 = denominators),
     one ScalarE copy, bf16 -> DRAM -> AllToAll.
  6. Each core normalizes its received token chunk (ln/exp reciprocal,
     batched gather + broadcast DMAs), output projection (i-outer matmuls),
     +bo, +residual, writes its [512, 512] column chunk of the output.
"""

import sys

sys.path.insert(0, "/opt/trn_rl_repo")

import numpy as np

import concourse.bacc as bacc
import concourse.tile as tile
from concourse import mybir
from concourse.bass_utils import run_bass_kernel_spmd

C = 512
N = 4096
HEADS = 8
D = 64
GROUPS = 32
CPG = C // GROUPS  # 16 channels per group
EPS = 1e-5
SCALE = D ** -0.5
NCORE = 8
NT = N // NCORE  # 512 tokens per core for the output projection
TQ = 512  # query-chunk (free dim of transposed scores); one psum bank
NTQ = N // TQ  # 8
TKC = 128  # key-chunk (partition dim of transposed scores)
NTK = N // TKC  # 32
CT = C // 128  # 4 channel tiles
DP = 80  # fp8 V stationary padded to 80 cols: col 64 = ones (denominator),
#          cols 65-79 = zero pad so the DoubleRow weights step is 16B-aligned

f32 = mybir.dt.float32
bf16 = mybir.dt.bfloat16
f8 = mybir.dt.float8e4
AF = mybir.ActivationFunctionType
ALU = mybir.AluOpType

_nc_cache = {}

# exp(SCALE*x) ~= ((x*EC0 + EC1)^2 + 0.5)^16  -- a (1 + y/16 + y^2/512)^16
# approximation computed in one fused VectorE pass (8 ALU stages), used to
# split softmax exp work between ScalarE and VectorE. Max rel err 2.9e-3 at
# |y|=1.6 (scores here stay well inside that), 3.5e-4 for |y|<0.8.
EC0 = SCALE / float(np.sqrt(512.0))
EC1 = float(np.sqrt(0.5))
SKEW_PAIRS = 4  # PV consumes pair j while scores/exp work on pair j+SKEW

# ACT/DVE exp split, one entry per 256-key PAIR: measured HW per-pair costs
# are near-equal (ScalarE ~1.68us vs VectorE-EXP16 ~1.72us incl. overhead),
# so strict alternation balances load and keeps consecutive pairs on
# different engines for maximum overlap.
EXP_DVE_PAT = tuple(j % 2 == 1 for j in range(NTK))


def _register_exp16():
    from concourse import dve_ops as dops
    from concourse.dve_spec import Spec, Src0, C0, C1, sq

    for op in dops.OPS:
        if op.name == "EXP16_ANT":
            return op
    t = sq(Src0 * C0 + C1) + C2_LEAF
    body = sq(sq(sq(sq(t))))
    spec = Spec(
        body=body,
        reference=lambda in0, in1, s0, s1, imm2: ((in0 * s0 + s1) ** 2 + imm2)
        ** 16,
    )
    op = dops.DveOp("EXP16_ANT", spec, subdim=False, uops_sha={})
    dops.OPS.append(op)
    dops.CUSTOM_DVE_SPECS[op.name] = op.spec
    dops._SUB_OPCODE_FOR_NAME[op.name] = dops._CUSTOM_DVE_ROW_BASE + len(dops.OPS) - 1
    from concourse.dve_uop import DveOpSpec
    from concourse.dve_spec import lower as dve_lower

    for ver in ("v3", "v4"):
        try:
            uops = dve_lower(spec, ver=ver)
            sha = DveOpSpec(
                name=op.name,
                opcode=dops.get_dve_sub_opcode(op.name),
                uops=uops,
                rd1_en=False,
            ).sha(ver)
            op.uops_sha[ver] = sha
        except Exception:
            pass
    return op


from concourse.dve_spec import C2 as C2_LEAF  # noqa: E402

EXP16 = _register_exp16()


def _attention_stage(nc, tc, ps_s, ps_o, pp, kTd, qTd, vpair_all, a2a_in):
    NPAIR = NTK // 2  # 16 pairs of 128-key chunks; PV runs fp8 DoubleRow
    PM = mybir.MatmulPerfMode.DoubleRow
    for jq in range(NTQ):
        # merged PV+den DoubleRow output: rows 0-63 = PV, row 64 = the ones
        # column denominator, rows 65-79 = padding (the fp8 weights are
        # padded to 80 cols so the dual-fp8 ldweights step%16==0 rule holds).
        ops = ps_o.tile([DP, TQ], f32, name="ops", tag="ops")
        p_tiles = {}

        sps_tiles = {}

        def mm_scores(tk):
            # pair-sized score tile: each 512-col matmul lands in one psum
            # bank; the pair is exp'd by a SINGLE instruction (per-instruction
            # overhead on HW is ~800ns, so fewer/bigger exps win).
            # K=64 contraction uses half the PE rows, so the even chunk runs
            # in row-group 0-1 (partitions 0-63) and the odd chunk runs
            # CONCURRENTLY in row-group 2-3 (partitions 64-127) via
            # tile_position row packing -- ~2x score throughput.
            j, half = tk // 2, tk % 2
            if half == 0:
                sps_tiles[j] = ps_s.tile(
                    [128, 2, TQ], f32, name="sps", tag="sps", bufs=3
                )
            sps = sps_tiles[j]
            base = 64 * half
            nc.tensor.matmul(
                sps[:, half, :],
                kTd[base : base + 64, j, :],
                qTd[base : base + 64, jq * TQ : (jq + 1) * TQ],
                start=True,
                stop=True,
            )
            if half == 0:
                return
            p_tiles[j] = pp.tile([128, 2, TQ], f8, name="p", tag="p", bufs=6)
            p = p_tiles[j]
            sps = sps_tiles.pop(j)
            if EXP_DVE_PAT[j % len(EXP_DVE_PAT)]:
                nc.vector._custom_dve(
                    EXP16, out=p[:, :, :], in0=sps[:, :, :],
                    s0=EC0, s1=EC1, imm2=0.5,
                )
            else:
                nc.scalar.activation(
                    p[:, :, :], sps[:, :, :], AF.Exp, scale=SCALE
                )

        def mm_pvden(j):
            p = p_tiles.pop(j)
            nc.tensor.matmul(
                ops[:, :],
                vpair_all[:, :, j, :],
                p[:, :, :],
                start=(j == 0),
                stop=(j == NPAIR - 1),
                perf_mode=PM,
            )

        # software pipeline: scores/exp run SKEW_PAIRS pair-steps ahead of
        # the PV accumulation so the PE never waits on a fresh exp.
        for tk in range(2 * SKEW_PAIRS):
            mm_scores(tk)
        for j in range(SKEW_PAIRS, NPAIR):
            mm_scores(2 * j)
            mm_scores(2 * j + 1)
            mm_pvden(j - SKEW_PAIRS)
        for j in range(NPAIR - SKEW_PAIRS, NPAIR):
            mm_pvden(j)

        # alternate the output copy between engines so neither exp stream
        # eats the full per-block copy cost (the 94.3us-attn configuration)
        o_sb = pp.tile([D + 1, TQ], bf16, name="o_sb", tag="o_sb", bufs=2)
        if jq % 2 == 0:
            nc.scalar.activation(o_sb[:, :], ops[0 : D + 1, :], AF.Copy)
        else:
            nc.vector.tensor_copy(o_sb[:, :], ops[0 : D + 1, :])
        nc.sync.dma_start(a2a_in[jq, :, :], o_sb[:, :])


def _build(attn_loop_k=None, pre_loop_k=None, tail_loop_k=None, pre_scope="all"):
    import contextlib
    nc = bacc.Bacc("TRN2", target_bir_lowering=False, debug=False, num_devices=NCORE)

    def scope_cm(tc, name):
        # For_i around just one pre sub-stage when pre_scope selects it
        if pre_loop_k and pre_scope == name:
            return tc.For_i(0, pre_loop_k, 1)
        return contextlib.nullcontext()

    x_d = nc.dram_tensor("x", [C, N], f32, kind="ExternalInput")
    gamma_d = nc.dram_tensor("gamma", [C, 1], f32, kind="ExternalInput")
    beta_d = nc.dram_tensor("beta", [C, 1], f32, kind="ExternalInput")
    wqT_d = nc.dram_tensor("wqT", [C, D], f32, kind="ExternalInput")
    wkT_d = nc.dram_tensor("wkT", [C, D], f32, kind="ExternalInput")
    # wv arrives zero-padded to [C, DP]: the v matmuls then emit the padded
    # 80-col layout directly and the ones (denominator) column comes from
    # the rank-1 bias matmul with vb[64] overwritten to 1.0.
    wvT_d = nc.dram_tensor("wvT", [C, DP], f32, kind="ExternalInput")
    woT_d = nc.dram_tensor("woT", [C, C], f32, kind="ExternalInput")
    bo_d = nc.dram_tensor("bo", [C, 1], f32, kind="ExternalInput")
    resid_d = nc.dram_tensor("resid", [C, NT], f32, kind="ExternalInput")
    bones_d = nc.dram_tensor("bones", [128, 8], f32, kind="ExternalInput")
    bonesT_d = nc.dram_tensor("bonesT", [8, 128], f32, kind="ExternalInput")
    out_d = nc.dram_tensor("out", [C, NT], f32, kind="ExternalOutput")

    with tile.TileContext(nc) as tc:
        with (
            tc.tile_pool(name="xc", bufs=1) as pxc,
            tc.tile_pool(name="xb", bufs=1) as pxb,
            tc.tile_pool(name="qk", bufs=1) as pqk,
            tc.tile_pool(name="vaug", bufs=1) as pva,
            tc.tile_pool(name="w", bufs=1) as pw,
            tc.tile_pool(name="small", bufs=1) as psm,
            tc.tile_pool(name="p", bufs=3) as pp,
            tc.tile_pool(name="post", bufs=1) as ppost,
            tc.tile_pool(name="dram", bufs=1, space="DRAM") as pdram,
        ):
            # ---------------- stage 1: load x + GroupNorm statistics ----------
            pre_cm = (
                tc.For_i(0, pre_loop_k, 1)
                if pre_loop_k and pre_scope == "all"
                else contextlib.nullcontext()
            )
            pre_cm.__enter__()
            s1_cm = scope_cm(tc, "s1")
            s1_cm.__enter__()
            # ---- x DMAs first (the long pole), params after ----------------
            xc = [pxc.tile([128, N], f32, name=f"xc{i}") for i in range(CT)]
            for i in range(CT):
                nc.sync.dma_start(xc[i][:, :], x_d[i * 128 : (i + 1) * 128, :])
            bones = psm.tile([128, 8], f32, name="bones")
            bonesT = psm.tile([8, 128], f32, name="bonesT")
            gamma_sb = psm.tile([128, CT], f32, name="gamma_sb")
            beta_sb = psm.tile([128, CT], f32, name="beta_sb")
            nc.sync.dma_start(bones[:, :], bones_d[:, :])
            nc.sync.dma_start(bonesT[:, :], bonesT_d[:, :])
            nc.sync.dma_start(
                gamma_sb[:, :], gamma_d.rearrange("(t p) o -> p (t o)", p=128)
            )
            nc.sync.dma_start(
                beta_sb[:, :], beta_d.rearrange("(t p) o -> p (t o)", p=128)
            )
            wq_sb = [pw.tile([128, D], f32, name=f"wq{i}") for i in range(CT)]
            wk_sb = [pw.tile([128, D], f32, name=f"wk{i}") for i in range(CT)]
            wv_sb = [pw.tile([128, DP], f32, name=f"wv{i}") for i in range(CT)]
            for i in range(CT):
                nc.sync.dma_start(wq_sb[i][:, :], wqT_d[i * 128 : (i + 1) * 128, :])
                nc.sync.dma_start(wk_sb[i][:, :], wkT_d[i * 128 : (i + 1) * 128, :])
                nc.sync.dma_start(wv_sb[i][:, :], wvT_d[i * 128 : (i + 1) * 128, :])

            # ---- per tile: raw bf16 cast (ACT) + channel stats (DVE) +
            # ---- per-tile group sums (PE): GN affine is folded into the
            # ---- projection weights later, so the cast needs no stats.
            xb = [pxb.tile([128, N], bf16, name=f"xb{i}") for i in range(CT)]
            cstat = [psm.tile([128, 2], f32, name=f"cs{i}") for i in range(CT)]
            gstat = psm.tile([8, 2, CT], f32, name="gstat")
            s_c = [psm.tile([128, 1], f32, name=f"s_c{i}") for i in range(CT)]
            b_c = [psm.tile([128, 1], f32, name=f"b_c{i}") for i in range(CT)]
            wqb = [pw.tile([128, D], bf16, name=f"wqb{i}") for i in range(CT)]
            wkb = [pw.tile([128, D], bf16, name=f"wkb{i}") for i in range(CT)]
            wvb = [pw.tile([128, DP], bf16, name=f"wvb{i}") for i in range(CT)]
            qb_sb = psm.tile([D, 1], f32, name="qb_sb")
            vb_sb = psm.tile([1, DP], bf16, name="vb_sb")
            ones_tok = psm.tile([1, 128], bf16, name="ones_tok")
            nc.vector.memset(ones_tok[:, :], 1.0)

            with tc.tile_pool(name="ps_pre", bufs=1, space="PSUM") as ps_pre:
                for i in range(CT):
                    # channel sums ride the bf16 cast for free (ACT accum);
                    # channel sum-of-squares is ONE big DVE pass per tile
                    # (bn_stats would need 8 instructions at the 512-free cap,
                    # and per-instruction overhead dominates on HW).
                    sx = psm.tile([128, 1], f32, name="sx", tag="sx", bufs=2)
                    nc.scalar.activation(
                        xb[i][:, :], xc[i][:, :], AF.Copy, accum_out=sx[:, :]
                    )
                    sq_scr = psm.tile([128, N], bf16, name="sq_scr", tag="sq", bufs=1)
                    sxx = psm.tile([128, 1], f32, name="sxx", tag="sxx", bufs=2)
                    nc.vector.scalar_tensor_tensor(
                        out=sq_scr[:, :], in0=xc[i][:, :], scalar=1.0,
                        in1=xc[i][:, :], op0=ALU.mult, op1=ALU.mult,
                        accum_out=sxx[:, :],
                    )
                    # cstat = (mean, E[x^2]) per channel
                    nc.vector.tensor_scalar_mul(cstat[i][:, 0:1], sx[:, :], 1.0 / N)
                    nc.vector.tensor_scalar_mul(cstat[i][:, 1:2], sxx[:, :], 1.0 / N)
                    gps = ps_pre.tile([8, 2], f32, name="gps", tag="gps", bufs=2)
                    nc.tensor.matmul(gps[:, :], bones[:, :], cstat[i][:, :])
                    nc.vector.tensor_copy(gstat[:, :, i], gps[:, :])

                # group mean/ex2 -> mean, rstd (layout [group8, stat2, tile4])
                gm = psm.tile([8, 2, CT], f32, name="gm")
                nc.vector.tensor_scalar_mul(gm[:, :, :], gstat[:, :, :], 1.0 / CPG)
                vtmp = psm.tile([8, 1, CT], f32, name="vtmp")
                nc.vector.tensor_mul(vtmp[:, 0, :], gm[:, 0, :], gm[:, 0, :])
                varg = psm.tile([8, 1, CT], f32, name="varg")
                nc.vector.tensor_sub(varg[:, 0, :], gm[:, 1, :], vtmp[:, 0, :])
                eps_sb = psm.tile([8, 1], f32, name="eps_sb")
                nc.vector.memset(eps_sb[:, :], EPS)
                lng = psm.tile([8, 1, CT], f32, name="lng")
                nc.scalar.activation(lng[:, 0, :], varg[:, 0, :], AF.Ln, bias=eps_sb[:, :])
                rstd = psm.tile([8, 1, CT], f32, name="rstd")
                nc.scalar.activation(rstd[:, 0, :], lng[:, 0, :], AF.Exp, scale=-0.5)
                gs2 = psm.tile([8, 2, CT], f32, name="gs2")
                nc.vector.tensor_copy(gs2[:, 0, :], gm[:, 0, :])
                nc.vector.tensor_copy(gs2[:, 1, :], rstd[:, 0, :])

                # expand group (mean, rstd) -> per-channel via bonesT matmul;
                # fold the GN affine into the projection weights
                for i in range(CT):
                    cbp = ps_pre.tile([128, 2], f32, name="cbp", tag="cbp", bufs=2)
                    nc.tensor.matmul(cbp[:, :], bonesT[:, :], gs2[:, :, i])
                    nc.vector.tensor_mul(s_c[i][:, :], cbp[:, 1:2], gamma_sb[:, i : i + 1])
                    nc.vector.tensor_mul(b_c[i][:, :], cbp[:, 0:1], s_c[i][:, :])
                    nc.vector.tensor_sub(b_c[i][:, :], beta_sb[:, i : i + 1], b_c[i][:, :])
                    for w_sb, wb in ((wq_sb, wqb), (wk_sb, wkb), (wv_sb, wvb)):
                        nc.vector.tensor_scalar(
                            out=wb[i][:, :],
                            in0=w_sb[i][:, :],
                            scalar1=s_c[i][:, :],
                            scalar2=None,
                            op0=ALU.mult,
                        )

                # bias vectors: qb = wq^T b (per-d, [D,1]); vb = b^T wv ([1,D]).
                # The K-side bias is dropped: its score contribution q.kb is
                # constant over the softmax (key) axis, so softmax(q.(k+kb))
                # == softmax(q.k) -- only the numerator/denominator scale by
                # the same per-query factor, which cancels.
                qbp = ps_pre.tile([D, 1], f32, name="qbp")
                vbp = ps_pre.tile([1, DP], f32, name="vbp")
                for i in range(CT):
                    nc.tensor.matmul(
                        qbp[:, :], wq_sb[i][:, :], b_c[i][:, :],
                        start=(i == 0), stop=(i == CT - 1),
                    )
                    nc.tensor.matmul(
                        vbp[:, :], b_c[i][:, :], wv_sb[i][:, :],
                        start=(i == 0), stop=(i == CT - 1),
                    )
                nc.vector.tensor_copy(qb_sb[:, :], qbp[:, :])
                nc.vector.tensor_copy(vb_sb[:, :], vbp[:, :])
                # vb col 64 = 1.0: the rank-1 bias matmul then emits the
                # ones (denominator) column of the padded v for every token.
                nc.vector.memset(vb_sb[0:1, D : D + 1], 1.0)

            s1_cm.__exit__(None, None, None)

            # ---------------- stage 3: q_T, k_T, v ----------------------------
            # Row-packed layouts for the tile_position score matmuls. Pair j
            # couples key chunk j (tokens [128j, 128j+128)) with chunk j+16
            # (second half of the token range) -- softmax is permutation-
            # invariant over keys, and this HALF-PAIRING makes every psum ->
            # SBUF copy contiguous AND partition-aligned:
            #   qTd: partitions 0-63 hold q^T, partitions 64-127 a duplicate
            #        (from the host-duplicated weight columns).
            #   kTd [128, pair, 128]: partitions 0-63 = chunks 0-15,
            #        partitions 64-127 = chunks 16-31.
            #   vpair_all [128, parity, pair, DP] fp8 (parity-major).
            kTd = pqk.tile([128, NTK // 2, TKC], bf16, name="kTd")
            qTd = pqk.tile([128, N], bf16, name="qTd")
            vpair_all = pva.tile([128, 2, NTK // 2, DP], f8, name="vpair")
            with (
                tc.tile_pool(name="ps_qk", bufs=2, space="PSUM") as ps_qk,
                tc.tile_pool(name="ps_v", bufs=2, space="PSUM") as ps_v,
            ):
                # paired psum tiles: one Identity+bias copy per 1024 tokens
                # (halves the copy instruction count; overhead-dominated on HW)
                qk_cm = scope_cm(tc, "qk")
                qk_cm.__enter__()
                for j2 in range(N // 1024):
                    pps = ps_qk.tile([D, 2, 512], f32, name="qps", tag="qkps")
                    for half in range(2):
                        j = 2 * j2 + half
                        for i in range(CT):
                            nc.tensor.matmul(
                                pps[:, half, :],
                                wqb[i][:, :],
                                xb[i][:, j * 512 : (j + 1) * 512],
                                start=(i == 0),
                                stop=(i == CT - 1),
                            )
                    nc.scalar.activation(
                        qTd[0:D, j2 * 1024 : (j2 + 1) * 1024], pps[:, :, :],
                        AF.Identity, bias=qb_sb[:, :],
                    )
                # duplicate q^T into partitions 64-127: one big contiguous
                # cross-partition DVE copy (bias already applied).
                nc.vector.tensor_copy(qTd[D : 2 * D, :], qTd[0:D, :])
                # k^T: no bias needed (see stage-2 comment). With half-
                # pairing, token groups 0-1 (chunks 0-15) fill kTd partitions
                # 0-63 and groups 2-3 (chunks 16-31) fill partitions 64-127;
                # all four are contiguous [64, 1024] copies (the latter two
                # shift partition base, which costs ~1.4x -- measured).
                for j2 in range(N // 1024):
                    pps = ps_qk.tile([D, 2, 512], f32, name="kps", tag="qkps")
                    for half in range(2):
                        j = 2 * j2 + half
                        for i in range(CT):
                            nc.tensor.matmul(
                                pps[:, half, :],
                                wkb[i][:, :],
                                xb[i][:, j * 512 : (j + 1) * 512],
                                start=(i == 0),
                                stop=(i == CT - 1),
                            )
                    base = 0 if j2 < 2 else D
                    slot = 8 * (j2 % 2)
                    if j2 % 2 == 0:
                        nc.scalar.activation(
                            kTd[base : base + D, slot : slot + 8, :],
                            pps[:, :, :],
                            AF.Copy,
                        )
                    else:
                        nc.vector.tensor_copy(
                            kTd[base : base + D, slot : slot + 8, :],
                            pps[:, :, :],
                        )
                qk_cm.__exit__(None, None, None)
                # v in [token, d] layout, fp8, already DP-padded (the zero
                # weight columns yield the pad; the rank-1 ones_tok x vb
                # matmul adds the folded GN bias AND writes the ones column
                # via vb[64] = 1). 4 token-chunks accumulate per one-bank
                # psum tile (4 x 320B = 1280B -- no matmul output crosses a
                # bank boundary) so one contiguous fp8 copy covers 4 slots.
                v_cm = scope_cm(tc, "v")
                v_cm.__enter__()
                for c4 in range(NTK // 4):
                    vps = ps_v.tile([128, 4, DP], f32, name="vps", tag="vps")
                    for sub in range(4):
                        jj = 4 * c4 + sub
                        for i in range(CT):
                            nc.tensor.matmul(
                                vps[:, sub, :],
                                xb[i][:, jj * 128 : (jj + 1) * 128],
                                wvb[i][:, :],
                                start=(i == 0),
                                stop=False,
                            )
                        nc.tensor.matmul(
                            vps[:, sub, :],
                            ones_tok[:, :],
                            vb_sb[:, :],
                            start=False,
                            stop=True,
                        )
                    nc.vector.tensor_copy(
                        vpair_all[:, c4 // 4, 4 * (c4 % 4) : 4 * (c4 % 4) + 4, :],
                        vps[:, :, :],
                    )
                v_cm.__exit__(None, None, None)

            pre_cm.__exit__(None, None, None)

            # ---------------- stage 4: attention ------------------------------
            a2a_in = pdram.tile([HEADS, D + 1, NT], bf16, name="a2a_in")
            with (
                tc.tile_pool(name="ps_s", bufs=3, space="PSUM") as ps_s,
                tc.tile_pool(name="ps_o", bufs=2, space="PSUM") as ps_o,
            ):
                import contextlib

                loop_cm = (
                    tc.For_i(
                        0,
                        attn_loop_k,
                        1,
                        hint_engines=(
                            mybir.EngineType.PE,
                            mybir.EngineType.Activation,
                        ),
                    )
                    if attn_loop_k
                    else contextlib.nullcontext()
                )
                with loop_cm:
                    _attention_stage(
                        nc, tc, ps_s, ps_o, pp, kTd, qTd, vpair_all, a2a_in
                    )

            # ---------------- stage 5: AllToAll -------------------------------
            wo_sb = [ppost.tile([128, C], f32, name=f"wo{i}") for i in range(4)]
            wob = [ppost.tile([128, C], bf16, name=f"wob{i}") for i in range(4)]
            resid_sb = [ppost.tile([128, NT], f32, name=f"res{i}") for i in range(CT)]
            bo_sb = ppost.tile([128, CT], f32, name="bo_sb")
            for i in range(4):
                nc.sync.dma_start(wo_sb[i][:, :], woT_d[i * 128 : (i + 1) * 128, :])
                nc.vector.tensor_copy(wob[i][:, :], wo_sb[i][:, :])
                nc.sync.dma_start(resid_sb[i][:, :], resid_d[i * 128 : (i + 1) * 128, :])
                nc.sync.dma_start(
                    bo_sb[:, i : i + 1], bo_d[i * 128 : (i + 1) * 128, :]
                )

            a2a_out = pdram.tile([HEADS, D + 1, NT], bf16, name="a2a_out")
            tail_cm = (
                tc.For_i(0, tail_loop_k, 1) if tail_loop_k else contextlib.nullcontext()
            )
            tail_cm.__enter__()
            nc.gpsimd.collective_compute(
                "AllToAll",
                ALU.bypass,
                replica_groups=[list(range(NCORE))],
                ins=[a2a_in.opt()],
                outs=[a2a_out.opt()],
            )

            # ---------------- stage 6: normalize + output projection ----------
            den = ppost.tile([HEADS, NT], bf16, name="den")
            nc.sync.dma_start(
                den[:, :],
                a2a_out[:, D, :],
            )
            dln = ppost.tile([HEADS, NT], f32, name="dln")
            nc.scalar.activation(dln[:, :], den[:, :], AF.Ln)
            drc = ppost.tile([HEADS, NT], f32, name="drc")
            nc.scalar.activation(drc[:, :], dln[:, :], AF.Exp, scale=-1.0)
            drc_dram = pdram.tile([HEADS, NT], f32, name="drc_dram")
            nc.sync.dma_start(drc_dram[:, :], drc[:, :])

            # batched gathers: all heads' unnormalized outputs in one DMA,
            # all heads' reciprocal denominators in one broadcast DMA
            rcv_all = ppost.tile([D, HEADS, NT], bf16, name="rcv_all")
            nc.sync.dma_start(
                rcv_all[:, :, :], a2a_out[:, 0:D, :].rearrange("h d t -> d h t")
            )
            bcr_all = ppost.tile([D, HEADS, NT], f32, name="bcr_all")
            nc.sync.dma_start(
                bcr_all[:, :, :],
                drc_dram.rearrange("h (o t) -> o (h t)", o=1).broadcast_to(
                    [D, HEADS * NT]
                ),
            )

            rhs_sb = [ppost.tile([128, NT], bf16, name=f"rhs{i}") for i in range(4)]
            for h in range(HEADS):
                nc.vector.tensor_mul(
                    rhs_sb[h // 2][(h % 2) * D : (h % 2) * D + D, :],
                    rcv_all[:, h, :],
                    bcr_all[:, h, :],
                )

            with tc.tile_pool(name="ps_y", bufs=1, space="PSUM") as ps_y:
                yps = [
                    ps_y.tile([128, NT], f32, name=f"yps{c}", tag=f"yps{c}")
                    for c in range(CT)
                ]
                for i in range(4):
                    for c in range(CT):
                        nc.tensor.matmul(
                            yps[c][:, :],
                            wob[i][:, c * 128 : (c + 1) * 128],
                            rhs_sb[i][:, :],
                            start=(i == 0),
                            stop=(i == 3),
                        )
                for c in range(CT):
                    y_sb = ppost.tile([128, NT], f32, name="y_sb", tag="y_sb", bufs=2)
                    nc.scalar.activation(
                        y_sb[:, :], yps[c][:, :], AF.Identity, bias=bo_sb[:, c : c + 1]
                    )
                    nc.vector.tensor_add(y_sb[:, :], y_sb[:, :], resid_sb[c][:, :])
                    nc.sync.dma_start(out_d[c * 128 : (c + 1) * 128, :], y_sb[:, :])

            tail_cm.__exit__(None, None, None)

    nc.compile()
    return nc


def get_nc():
    if "nc" not in _nc_cache:
        _nc_cache["nc"] = _build()
    return _nc_cache["nc"]


def make_in_maps(hidden_states, gn_gamma, gn_beta, wq, wk, wv, wo, bo):
    x2d = np.ascontiguousarray(
        np.asarray(hidden_states, dtype=np.float32).reshape(C, N)
    )
    gamma = np.ascontiguousarray(np.asarray(gn_gamma, np.float32).reshape(C, 1))
    beta = np.ascontiguousarray(np.asarray(gn_beta, np.float32).reshape(C, 1))
    wq = np.asarray(wq, np.float32)
    wk = np.asarray(wk, np.float32)
    wv = np.asarray(wv, np.float32)
    woT = np.ascontiguousarray(np.asarray(wo, np.float32).T)
    bo2 = np.ascontiguousarray(np.asarray(bo, np.float32).reshape(C, 1))
    bones = np.zeros((128, 8), np.float32)
    for cc in range(128):
        bones[cc, cc // CPG] = 1.0
    bonesT = np.ascontiguousarray(bones.T)
    in_maps = []
    for h in range(NCORE):
        sl = slice(h * D, (h + 1) * D)
        in_maps.append(
            {
                "x": x2d,
                "gamma": gamma,
                "beta": beta,
                "wqT": np.ascontiguousarray(wq[sl, :].T),
                "wkT": np.ascontiguousarray(wk[sl, :].T),
                "wvT": np.ascontiguousarray(
                    np.concatenate(
                        [wv[sl, :].T, np.zeros((C, DP - D), np.float32)], axis=1
                    )
                ),
                "woT": woT,
                "bo": bo2,
                "resid": np.ascontiguousarray(x2d[:, h * NT : (h + 1) * NT]),
                "bones": bones,
                "bonesT": bonesT,
            }
        )
    return in_maps


def kernel(hidden_states, gn_gamma, gn_beta, wq, wk, wv, wo, bo):
    nc = get_nc()
    in_maps = make_in_maps(hidden_states, gn_gamma, gn_beta, wq, wk, wv, wo, bo)
    res = run_bass_kernel_spmd(nc, in_maps, core_ids=list(range(NCORE)))
    out2d = np.empty((C, N), np.float32)
    for h in range(NCORE):
        out2d[:, h * NT : (h + 1) * NT] = res.results[h]["out"]
    return out2d.reshape(1, C, 64, 64)

